# revision 33
# baseline (speedup 1.0000x reference)
"""Trainium2 Bass kernel for EnhancedSelfAttention (GroupNorm + MHSA + proj + residual).

Problem: x[16, 256, 32, 32] f32; GroupNorm(1 group) -> 1x1-conv qkv (768x256)
-> 8-head self-attention over the 1024 pixels (head_dim 32) -> 1x1-conv proj
(256x256) -> + x.

Strategy: pure data parallelism over the batch, 2 samples per NeuronCore on 8
cores, no collectives. Per sample, everything stays on-chip:

  - GroupNorm stats via bn_stats/bn_aggr + a ones-matmul partition reduction;
    rsqrt on DVE (Newton, no ACT table switch - ACT does exp exclusively).
  - qkv as matmuls with channels on partitions: q^T,k^T = W_qk^T.T @ xn,
    v in natural [pixel, ch] layout via v = xn.T @ W_v^T (second matmul pass).
  - Attention per head in the transposed layout S^T[key, query] so softmax's
    sum lands on the matmul contraction: O_unnorm^T = v_ones.T @ exp(S^T),
    where v_ones carries an extra ones column producing the softmax
    denominator in the same stream (row 32 of the PSUM accumulator). exp() has
    no max-subtraction: |S*scale| stays < ~7 so fp32 exp cannot overflow.
  - Normalization: reciprocal of the denominator row, broadcast across
    partitions with an indicator-matmul, one multiply on PSUM evacuation.
  - proj + bias + residual fused into the PSUM evacuation op.

Performance notes (measured on TRN2):
  - K=32 row-tiled matmuls never register as PE activity, so the HAM clock
    gate stays at 4/8 (1.2 GHz) forever. The S^T matmuls are therefore K=128
    with a zero-padded per-band copy of k^T (zeros annihilate the other
    heads' q rows, which ride along in the dense moving operand).
  - Matmul operands are bf16 (fp32r is a 2-pass mode whose weight loads
    never overlap; bf16 end-to-end error here is ~6e-4 relative, measured).
    PSUM accumulation is fp32 throughout. The pair's two O matmuls are
    column-tiled into one PSUM tile (col groups 0/64) and run concurrently.
  - The emission schedule interleaves each chunk's softmax-normalization and
    the projection with the following attention pairs so neither the DVE
    FIFO nor the PE FIFO head-of-line blocks; exp() owns the Scalar engine
    (GroupNorm rsqrt is a DVE Newton iteration, avoiding an ACT table swap).
"""

import sys

import ml_dtypes
import numpy as np

for _p in ("/opt/trn_rl_repo",):
    if _p not in sys.path:
        sys.path.insert(0, _p)

import concourse.bass as bass  # noqa: F401
import concourse.tile as tile
from concourse import bacc, mybir
from concourse.bass_utils import run_bass_kernel_spmd

BF16 = mybir.dt.bfloat16

F32 = mybir.dt.float32
F32R = mybir.dt.float32r
I32 = mybir.dt.int32
AF = mybir.ActivationFunctionType
OP = mybir.AluOpType

B, C, HW = 16, 256, 1024
NH, HD = 8, 32
NCORES = 8
SPC = B // NCORES  # samples per core
EPS = 1e-5
SCALE = float(HD) ** -0.5

_CACHE: dict = {}

_IND4 = np.zeros((128, 128), np.float32)
for _i in range(4):
    _IND4[_i, 32 * _i : 32 * _i + 32] = 1.0


def _emit_gn(nc, pools, x_sb, xn_sb, gnw_sb, gnb_sb, ones_col, ones_row):
    """GroupNorm(1 group) over the full [256, 1024] sample."""
    tp, ps_misc = pools["t"], pools["ps_misc"]
    # per-partition stats over both channel chunks (2048 elems per partition)
    stat6 = tp.tile([128, 4, 6], F32, tag="stat6")
    for i in range(4):
        nc.vector.bn_stats(
            out=stat6[:, i, :], in_=x_sb[:, i // 2, 512 * (i % 2) : 512 * (i % 2) + 512]
        )
    mv = tp.tile([128, 2], F32, tag="mv")
    nc.vector.bn_aggr(out=mv, in_=stat6)
    # st2 = [mean_p, E[x^2]_p]
    st2 = tp.tile([128, 2], F32, tag="st2")
    nc.vector.tensor_copy(out=st2[:, 0:1], in_=mv[:, 0:1])
    nc.vector.scalar_tensor_tensor(
        out=st2[:, 1:2],
        in0=mv[:, 0:1],
        scalar=mv[:, 0:1],
        in1=mv[:, 1:2],
        op0=OP.mult,
        op1=OP.add,
    )
    # partition reduction: [1, 2] = ones.T @ st2
    ps_g = ps_misc.tile([1, 2], F32, tag="st")
    nc.tensor.matmul(out=ps_g, lhsT=ones_col, rhs=st2, start=True, stop=True)
    # scalars: g = (mean, E[x^2]); var = e2 - mean^2; rstd = rsqrt(var + eps)
    sc = tp.tile([1, 8], F32, tag="sc")
    nc.vector.tensor_scalar_mul(out=sc[:, 0:2], in0=ps_g, scalar1=1.0 / 128.0)
    nc.vector.tensor_mul(out=sc[:, 2:3], in0=sc[:, 0:1], in1=sc[:, 0:1])
    nc.vector.tensor_sub(out=sc[:, 3:4], in0=sc[:, 1:2], in1=sc[:, 2:3])
    nc.vector.tensor_scalar_add(out=sc[:, 4:5], in0=sc[:, 3:4], scalar1=EPS)
    vep = sc[:, 4:5]
    # Newton rsqrt seeded by the classic bit trick (robust to any scale)
    yt = tp.tile([1, 8], F32, tag="yt")
    nc.vector.tensor_scalar(
        out=yt[:, 0:1].bitcast(I32),
        in0=vep.bitcast(I32),
        scalar1=1,
        scalar2=None,
        op0=OP.logical_shift_right,
    )
    nc.vector.tensor_scalar(
        out=yt[:, 1:2].bitcast(I32),
        in0=yt[:, 0:1].bitcast(I32),
        scalar1=-1,
        scalar2=0x5F3759DF,
        op0=OP.mult,
        op1=OP.add,
    )
    y = yt[:, 1:2]
    for it in range(3):
        t0 = yt[:, 2 + it : 3 + it] if it < 2 else yt[:, 2 + (it % 2) : 3 + (it % 2)]
        nc.vector.tensor_mul(out=t0, in0=vep, in1=y)
        nc.vector.tensor_mul(out=t0, in0=t0, in1=y)
        nc.vector.tensor_scalar(
            out=t0, in0=t0, scalar1=-0.5, scalar2=1.5, op0=OP.mult, op1=OP.add
        )
        ynew = yt[:, 4 + (it % 2) : 5 + (it % 2)]
        nc.vector.tensor_mul(out=ynew, in0=y, in1=t0)
        y = ynew
    # fin = [-mean, rstd]
    fin = tp.tile([1, 2], F32, tag="fin")
    nc.vector.tensor_scalar_mul(out=fin[:, 0:1], in0=sc[:, 0:1], scalar1=-1.0)
    nc.vector.tensor_copy(out=fin[:, 1:2], in_=y)
    # broadcast to all 128 partitions
    ps_b = ps_misc.tile([128, 2], F32, tag="st")
    nc.tensor.matmul(out=ps_b, lhsT=ones_row, rhs=fin, start=True, stop=True)
    bc = tp.tile([128, 2], F32, tag="bc")
    nc.vector.tensor_copy(out=bc, in_=ps_b)
    # affine: xn = x * (rstd*gn_w) + (gn_b - mean*rstd*gn_w)
    a_sb = tp.tile([128, 2], F32, tag="asb")
    nc.vector.tensor_scalar_mul(out=a_sb, in0=gnw_sb, scalar1=bc[:, 1:2])
    b_sb = tp.tile([128, 2], F32, tag="bsb")
    nc.vector.scalar_tensor_tensor(
        out=b_sb, in0=a_sb, scalar=bc[:, 0:1], in1=gnb_sb, op0=OP.mult, op1=OP.add
    )
    for k in range(2):
        nc.vector.tensor_scalar(
            out=xn_sb[:, k, :],
            in0=x_sb[:, k, :],
            scalar1=a_sb[:, k : k + 1],
            scalar2=b_sb[:, k : k + 1],
            op0=OP.mult,
            op1=OP.add,
        )


def _build():
    nc = bacc.Bacc("TRN2", target_bir_lowering=False, debug=False)
    x_d = nc.dram_tensor("x", [SPC, C, HW], F32, kind="ExternalInput").ap()
    qkvwT_d = nc.dram_tensor("qkv_wT", [C, 3 * C], BF16, kind="ExternalInput").ap()
    qbqk_d = nc.dram_tensor("qkv_b_qk", [4, 128], F32, kind="ExternalInput").ap()
    qbv_d = nc.dram_tensor("qkv_b_v", [1, C], F32, kind="ExternalInput").ap()
    pwT_d = nc.dram_tensor("proj_wT", [C, C], BF16, kind="ExternalInput").ap()
    pb_d = nc.dram_tensor("proj_b", [2, 128], F32, kind="ExternalInput").ap()
    gnw_d = nc.dram_tensor("gn_w", [2, 128], F32, kind="ExternalInput").ap()
    gnb_d = nc.dram_tensor("gn_b", [2, 128], F32, kind="ExternalInput").ap()
    ind4_d = nc.dram_tensor("ind4", [128, 128], BF16, kind="ExternalInput").ap()
    out_d = nc.dram_tensor("out", [SPC, C, HW], F32, kind="ExternalOutput").ap()

    with tile.TileContext(nc) as tc:
        _emit(
            nc, tc, x_d, qkvwT_d, qbqk_d, qbv_d, pwT_d, pb_d, gnw_d, gnb_d, ind4_d,
            out_d,
        )
    nc.compile()
    return nc


def _emit(
    nc, tc, x_d, qkvwT_d, qbqk_d, qbv_d, pwT_d, pb_d, gnw_d, gnb_d, ind4_d, out_d
):
    from contextlib import ExitStack

    with ExitStack() as ctx:
        singles = ctx.enter_context(tc.tile_pool(name="singles", bufs=1))
        samp = ctx.enter_context(tc.tile_pool(name="samp", bufs=2))
        tp = ctx.enter_context(tc.tile_pool(name="small", bufs=3))
        e_pool = ctx.enter_context(tc.tile_pool(name="epool", bufs=4))
        ps_st = ctx.enter_context(tc.tile_pool(name="ps_st", bufs=2, space="PSUM"))
        ps_o = ctx.enter_context(tc.tile_pool(name="ps_o", bufs=2, space="PSUM"))
        ps_misc = ps_st  # transient matmul psums share the S^T slots (tag "st")
        pools = {"t": tp, "ps_misc": ps_misc}

        x_tiles = []
        for s in range(SPC):
            x_sb = samp.tile([128, 2, HW], F32, name="x_sb", tag="x")
            nc.sync.dma_start(
                out=x_sb, in_=x_d[s].rearrange("(k p) n -> p k n", p=128)
            )
            x_tiles.append(x_sb)

        # ---- kernel-lifetime constants ----
        qkvwT = singles.tile([128, 2, 3 * C], BF16)
        nc.sync.dma_start(
            out=qkvwT, in_=qkvwT_d.rearrange("(k p) o -> p k o", p=128)
        )
        pwT = singles.tile([128, 2, C], BF16)
        nc.sync.dma_start(
            out=pwT, in_=pwT_d.rearrange("(k p) o -> p k o", p=128)
        )
        qb_sb = singles.tile([128, 4], F32)
        nc.sync.dma_start(out=qb_sb, in_=qbqk_d.rearrange("t p -> p t"))
        pb_sb = singles.tile([128, 2], F32)
        nc.sync.dma_start(out=pb_sb, in_=pb_d.rearrange("t p -> p t"))
        gnw_sb = singles.tile([128, 2], F32)
        nc.sync.dma_start(out=gnw_sb, in_=gnw_d.rearrange("t p -> p t"))
        gnb_sb = singles.tile([128, 2], F32)
        nc.sync.dma_start(out=gnb_sb, in_=gnb_d.rearrange("t p -> p t"))
        qbv_sb = singles.tile([1, C], F32)
        nc.sync.dma_start(out=qbv_sb, in_=qbv_d)
        ind4_sb = singles.tile([128, 128], BF16)
        nc.sync.dma_start(out=ind4_sb, in_=ind4_d)
        zeros_col = singles.tile([128, 1], F32)
        nc.vector.memset(zeros_col, 0.0)
        kpad = []
        for i in range(4):
            kp = singles.tile([128, HW], BF16, name=f"kpad{i}")
            nc.vector.tensor_copy(out=kp, in_=zeros_col.to_broadcast([128, HW]))
            kpad.append(kp)
        ones_col = singles.tile([128, 1], F32)
        nc.vector.memset(ones_col, 1.0)
        ones_row = singles.tile([1, 128], F32)
        nc.vector.memset(ones_row, 1.0)
        # dummy exp: pulls the ~2.7us ACT table load off the critical path
        dummy_e = tp.tile([1, 8], F32, name="dummy_e", tag="de")
        nc.scalar.activation(out=dummy_e, in_=ones_row[:, 0:8], func=AF.Exp, scale=0.01)
        # dummy bf16 matmul burst: pre-warms the HAM clock gate during GN
        db = singles.tile([128, 512], BF16)
        nc.vector.memset(db, 0.5)
        for _i in range(16):
            pd = ps_misc.tile([64, 512], F32, name="pd", tag="st")
            nc.tensor.matmul(
                out=pd,
                lhsT=db[:, 0:64],
                rhs=db,
                start=True,
                stop=True,
                skip_group_check=True,
            )
        # broadcast of the v-part qkv bias along partitions: [128, 256]
        vb_ps = ps_misc.tile([128, C], F32, tag="st")
        nc.tensor.matmul(out=vb_ps, lhsT=ones_row, rhs=qbv_sb, start=True, stop=True)
        vb_bc = singles.tile([128, C], F32)
        nc.vector.tensor_copy(out=vb_bc, in_=vb_ps)

        xn_tiles = []
        for s in range(SPC):
            xn_sb = samp.tile([128, 2, HW], BF16, name="xn_sb", tag="xn")
            _emit_gn(
                nc, pools, x_tiles[s], xn_sb, gnw_sb, gnb_sb, ones_col, ones_row
            )
            xn_tiles.append(xn_sb)

        def emit_qkv(s):
            xn_sb = xn_tiles[s]
            qk_sb = samp.tile([128, 4, HW], BF16, name="qk_sb", tag="qk")
            for mt in range(4):
                for hf in range(2):
                    ps = ps_misc.tile([128, 512], F32, name="ps_q", tag="st")
                    for kc in range(2):
                        nc.tensor.matmul(
                            out=ps,
                            lhsT=qkvwT[:, kc, 128 * mt : 128 * mt + 128],
                            rhs=xn_sb[:, kc, 512 * hf : 512 * hf + 512],
                            start=(kc == 0),
                            stop=(kc == 1),
                            skip_group_check=True,
                        )
                    nc.vector.tensor_scalar_add(
                        out=qk_sb[:, mt, 512 * hf : 512 * hf + 512],
                        in0=ps,
                        scalar1=qb_sb[:, mt : mt + 1],
                    )
            vn_sb = samp.tile([128, 8, NH, HD + 1], BF16, name="vn_sb", tag="vn")
            nc.vector.tensor_copy(
                out=vn_sb[:, :, :, HD : HD + 1],
                in_=ones_col.to_broadcast([128, 8, NH, 1]),
            )
            for j in range(8):
                ps = ps_misc.tile([128, C], F32, name="ps_v", tag="st")
                for kc in range(2):
                    nc.tensor.matmul(
                        out=ps,
                        lhsT=xn_sb[:, kc, 128 * j : 128 * j + 128],
                        rhs=qkvwT[:, kc, 2 * C : 3 * C],
                        start=(kc == 0),
                        stop=(kc == 1),
                        skip_group_check=True,
                    )
                nc.vector.tensor_add(
                    out=vn_sb[:, j, :, 0:HD],
                    in0=ps.rearrange("p (h d) -> p h d", h=NH),
                    in1=vb_bc.rearrange("p (h d) -> p h d", h=NH),
                )
            return qk_sb, vn_sb

        state = {}

        def emit_pair(s, pr):
            qk_sb, vn_sb, o32_sb, rs_raw, rsis, pps = state[s]
            heads = (2 * pr, 2 * pr + 1)
            chunk = pr // 2
            if pr % 2 == 0:
                rs_raw[chunk] = tp.tile(
                    [4, HW], F32, name=f"rsr{chunk}", tag=f"rsr{chunk}"
                )
            for h in heads:
                qbase = 32 * (h % 4)
                mk = 2 + h // 4
                nc.sync.dma_start(
                    out=kpad[h % 4][qbase : qbase + 32, :],
                    in_=qk_sb[qbase : qbase + 32, mk, :],
                )
            o_ps = ps_o.tile([128, HW], F32, name="o_ps", tag="o")
            st_tiles = {}
            for j in range(8):
                for h in heads:
                    mq = h // 4
                    st = ps_st.tile([128, HW], F32, name="st", tag="st")
                    st_tiles[h] = st
                    for hf in range(2):
                        nc.tensor.matmul(
                            out=st[:, 512 * hf : 512 * hf + 512],
                            lhsT=kpad[h % 4][:, 128 * j : 128 * j + 128],
                            rhs=qk_sb[:, mq, 512 * hf : 512 * hf + 512],
                            start=True,
                            stop=True,
                            skip_group_check=True,
                        )
                for t, h in enumerate(heads):
                    e = e_pool.tile([128, HW], BF16, name="e", tag="e")
                    nc.scalar.activation(
                        out=e, in_=st_tiles[h], func=AF.Exp, scale=SCALE
                    )
                    cg = 64 * t  # column group: head A rows 0-32, head B 64-96
                    for hf in range(2):
                        nc.tensor.matmul(
                            out=o_ps[cg : cg + 33, 512 * hf : 512 * hf + 512],
                            lhsT=vn_sb[:, j, h, :],
                            rhs=e[:, 512 * hf : 512 * hf + 512],
                            start=(j == 0),
                            stop=(j == 7),
                            tile_position=(0, cg),
                            skip_group_check=True,
                        )
            # unnormalized evacuation (on ACT, which has slack) + denom stash
            for t, h in enumerate(heads):
                qbase = 32 * (h % 4)
                mq = h // 4
                cg = 64 * t
                nc.vector.tensor_copy(
                    out=o32_sb[qbase : qbase + 32, mq, :],
                    in_=o_ps[cg : cg + 32, :],
                )
                den_sb = tp.tile([1, HW], F32, name="den_sb", tag="den")
                nc.vector.tensor_copy(out=den_sb, in_=o_ps[cg + 32 : cg + 33, :])
                nc.sync.dma_start(
                    out=rs_raw[chunk][h % 4 : h % 4 + 1, :], in_=den_sb
                )

        def emit_norm_recip(s, chunk):
            qk_sb, vn_sb, o32_sb, rs_raw, rsis, pps = state[s]
            rsi = samp.tile([128, HW], BF16, name="rsi", tag=f"rsi{chunk}")
            rsis[chunk] = rsi
            nc.vector.tensor_copy(
                out=rsi, in_=zeros_col.to_broadcast([128, HW])
            )
            with nc.allow_low_precision(reason="f32r is fp32-class"):
                for hf in range(2):
                    nc.vector.reciprocal(
                        out=rsi[0:4, 512 * hf : 512 * hf + 512],
                        in_=rs_raw[chunk][:, 512 * hf : 512 * hf + 512],
                    )

        def emit_norm_apply(s, chunk, halves=(0, 1)):
            qk_sb, vn_sb, o32_sb, rs_raw, rsis, pps = state[s]
            rsi = rsis[chunk]
            for hf in halves:
                bc = ps_misc.tile([128, 512], F32, name="bc", tag="st")
                nc.tensor.matmul(
                    out=bc,
                    lhsT=ind4_sb,
                    rhs=rsi[:, 512 * hf : 512 * hf + 512],
                    start=True,
                    stop=True,
                    skip_group_check=True,
                )
                bc_sb = tp.tile([128, 512], F32, name="bc_sb", tag="bcs")
                nc.vector.tensor_copy(out=bc_sb, in_=bc)
                nc.vector.tensor_mul(
                    out=o32_sb[:, chunk, 512 * hf : 512 * hf + 512],
                    in0=o32_sb[:, chunk, 512 * hf : 512 * hf + 512],
                    in1=bc_sb,
                )

        def emit_proj(s):
            qk_sb, vn_sb, o32_sb, rs_raw, rsis, pps = state[s]
            x_sb = x_tiles[s]
            out_sb = samp.tile([128, 2, HW], F32, name="out_sb", tag="outsb")
            for mt in range(2):
                for hf in range(2):
                    ps = ps_misc.tile([128, 512], F32, name="ps_p", tag="st")
                    for kc in range(2):
                        nc.tensor.matmul(
                            out=ps,
                            lhsT=pwT[:, kc, 128 * mt : 128 * mt + 128],
                            rhs=o32_sb[:, kc, 512 * hf : 512 * hf + 512],
                            start=(kc == 0),
                            stop=(kc == 1),
                            skip_group_check=True,
                        )
                    nc.vector.scalar_tensor_tensor(
                        out=out_sb[:, mt, 512 * hf : 512 * hf + 512],
                        in0=ps,
                        scalar=pb_sb[:, mt : mt + 1],
                        in1=x_sb[:, mt, 512 * hf : 512 * hf + 512],
                        op0=OP.add,
                        op1=OP.add,
                    )
                    nc.sync.dma_start(
                        out=out_d[s].rearrange("(k p) n -> p k n", p=128)[
                            :, mt, 512 * hf : 512 * hf + 512
                        ],
                        in_=out_sb[:, mt, 512 * hf : 512 * hf + 512],
                    )

        for s in range(SPC):
            qk_sb, vn_sb = emit_qkv(s)
            o32_sb = samp.tile([128, 2, HW], BF16, name="o32_sb", tag="o32")
            state[s] = (qk_sb, vn_sb, o32_sb, [None, None], [None, None], {})

        # interleaved schedule: the normalize chain of a chunk is spread over
        # the following pairs so neither the DVE FIFO nor the PE FIFO blocks
        emit_pair(0, 0)
        emit_pair(0, 1)
        emit_pair(0, 2)
        emit_norm_recip(0, 0)
        emit_pair(0, 3)
        emit_norm_apply(0, 0)
        emit_norm_recip(0, 1)
        emit_pair(1, 0)
        emit_norm_apply(0, 1)
        emit_pair(1, 1)
        emit_proj(0)
        emit_pair(1, 2)
        emit_norm_recip(1, 0)
        emit_pair(1, 3)
        emit_norm_recip(1, 1)
        emit_norm_apply(1, 0)
        emit_norm_apply(1, 1)
        emit_proj(1)
def _get_nc():
    if "nc" not in _CACHE:
        _CACHE["nc"] = _build()
    return _CACHE["nc"]


def kernel(x, gn_w, gn_b, qkv_w, qkv_b, proj_w, proj_b, **_ignored):
    nc = _get_nc()
    x = np.asarray(x, dtype=np.float32).reshape(B, C, HW)
    qkv_wT = np.ascontiguousarray(
        np.asarray(qkv_w, np.float32).T.astype(ml_dtypes.bfloat16)
    )
    proj_wT = np.ascontiguousarray(
        np.asarray(proj_w, np.float32).T.astype(ml_dtypes.bfloat16)
    )
    qkv_b = np.asarray(qkv_b, np.float32)
    shared = {
        "qkv_wT": qkv_wT,
        "qkv_b_qk": np.ascontiguousarray(qkv_b[: 2 * C].reshape(4, 128)),
        "qkv_b_v": np.ascontiguousarray(qkv_b[2 * C :].reshape(1, C)),
        "proj_wT": proj_wT,
        "proj_b": np.ascontiguousarray(np.asarray(proj_b, np.float32).reshape(2, 128)),
        "gn_w": np.ascontiguousarray(np.asarray(gn_w, np.float32).reshape(2, 128)),
        "gn_b": np.ascontiguousarray(np.asarray(gn_b, np.float32).reshape(2, 128)),
        "ind4": _IND4.astype(ml_dtypes.bfloat16),
    }
    in_maps = [
        {"x": np.ascontiguousarray(x[i * SPC : (i + 1) * SPC]), **shared}
        for i in range(NCORES)
    ]
    br = run_bass_kernel_spmd(nc, in_maps, core_ids=list(range(NCORES)))
    out = np.concatenate([r["out"] for r in br.results], axis=0)
    return out.reshape(B, C, 32, 32)
